# revision 1
# baseline (speedup 1.0000x reference)
"""DeeperGCN layer as a Bass/Tile kernel for TRN2, 8-core SPMD.

Sharding: nodes are partitioned across 8 cores (dst-sharded), with a
degree-balanced node->tile assignment so every (core, tile) bin holds a
near-equal number of edges (minimizes padded edge capacity). Host pre-sorts
edges by destination into per-(core, node-tile) padded bins, so every core's
segment-softmax reductions are fully local. The only collective is one
AllGather of the BN+ReLU'd node features (hn), which every core needs as
gather source for its edges' src nodes.

Per node-tile (128 dst nodes, capacity NCHUNK*128 edges):
  - gather hn[src] rows via SWDGE indirect DMA (4 queues, round-robin)
  - s = hn_src + eattr  (bf16, DVE/GpSimd split)
  - ex = max(exp(t*s + t*eps), exp(t*eps))  == exp(t*(relu(s)+eps))
  - mex = relu(s)*ex    (the +eps part of the message is folded in later:
                         num = mex_sum + eps*den)
  - indicator A[e, n] = (dst_slot[e] == n) built on-device vs iota
  - one matmul per 128-edge chunk accumulates [den|num'] in PSUM
  - agg = (num' + eps*den)/(den+1e-16); x_r = agg + hn_row (hn rows held in
    SBUF from phase A); MLP (2 GEMMs + LayerNorm); out = h + mlp_out
"""

import dataclasses
import numpy as np

import concourse.bass as bass
import concourse.bacc as bacc
import concourse.tile as tile
import concourse.mybir as mybir
from concourse.masks import make_identity

F32 = mybir.dt.float32
F32R = mybir.dt.float32r
BF16 = mybir.dt.bfloat16
I32 = mybir.dt.int32
I16 = mybir.dt.int16
AF = mybir.ActivationFunctionType
OP = mybir.AluOpType

EPS = 1e-7
BN_EPS = 1e-5
LN_EPS = 1e-5
DEN_EPS = 1e-16


@dataclasses.dataclass
class Cfg:
    n_cores: int = 8
    H: int = 128
    NT: int = 49          # node tiles per core
    C_LO: int = 11        # chunks of src<SPLIT edges per tile
    C_HI: int = 6         # chunks of src>=SPLIT edges per tile
    skip_collective: bool = False  # replace AllGather with local copy
    n_queues: int = 4     # SWDGE queues for gather round-robin
    hw_repeat: int = 1    # repeat phase B (timing amplification)
    stream_bufs: int = 3
    small_bufs: int = 3
    add_gp_chunks: int = 0    # chunks of the s=hs+ea add done on gpsimd
    mex_gp_chunks: int = 0    # chunks of the mex STT done on gpsimd
    iseq_gp_chunks: int = 0   # chunks of the indicator build done on gpsimd
    iseq_dve_ts: bool = True  # DVE indicator via per-chunk tensor_scalar
    fat_gidx: bool = False    # send gather idxs pre-replicated to 128 parts
    apply_b1: bool = False
    apply_b2: bool = False
    apply_ln_affine: bool = False
    ablate: str = ""

    SPLIT: int = 32768

    @property
    def NCHUNK(self):
        return self.C_LO + self.C_HI

    @property
    def NPC(self):
        return self.NT * 128

    @property
    def NP(self):
        return self.NPC * self.n_cores

    @property
    def H2(self):
        return 2 * self.H


def build_gcn(cfg: Cfg):
    H, H2, NT, NCHUNK = cfg.H, cfg.H2, cfg.NT, cfg.NCHUNK
    NPC, NP = cfg.NPC, cfg.NP
    CE = NCHUNK * 128  # edge capacity per tile
    sdt = BF16

    nc = bacc.Bacc("TRN2", target_bir_lowering=False, debug=False,
                   num_devices=cfg.n_cores, num_swdge_queues=cfg.n_queues)

    # ---- I/O ----
    h_rows = nc.dram_tensor("h_rows", [NPC, H], F32, kind="ExternalInput").ap()
    bnw = nc.dram_tensor("bnw", [1, H], F32, kind="ExternalInput").ap()
    bnb = nc.dram_tensor("bnb", [1, H], F32, kind="ExternalInput").ap()
    bnm = nc.dram_tensor("bnm", [1, H], F32, kind="ExternalInput").ap()
    bnv = nc.dram_tensor("bnv", [1, H], F32, kind="ExternalInput").ap()
    t_sc = nc.dram_tensor("t_sc", [1, 1], F32, kind="ExternalInput").ap()
    W1 = nc.dram_tensor("W1", [H, H2], F32, kind="ExternalInput").ap()
    W2 = nc.dram_tensor("W2", [H2, H], F32, kind="ExternalInput").ap()
    if cfg.apply_b1:
        b1 = nc.dram_tensor("b1", [1, H2], F32, kind="ExternalInput").ap()
    if cfg.apply_b2:
        b2 = nc.dram_tensor("b2", [1, H], F32, kind="ExternalInput").ap()
    if cfg.apply_ln_affine:
        lnw = nc.dram_tensor("lnw", [1, H2], F32, kind="ExternalInput").ap()
        lnb = nc.dram_tensor("lnb", [1, H2], F32, kind="ExternalInput").ap()
    if cfg.fat_gidx:
        gidx = nc.dram_tensor("gidx", [NT, 128, CE // 16], I16,
                              kind="ExternalInput").ap()
    else:
        gidx = nc.dram_tensor("gidx", [NT, 16, 2 * (CE // 16)], I16,
                              kind="ExternalInput").ap()
    dst_sl = nc.dram_tensor("dst_sl", [NT, 128, NCHUNK], F32,
                            kind="ExternalInput").ap()
    eattr = nc.dram_tensor("eattr", [NT, 128, CE], sdt,
                           kind="ExternalInput").ap()
    out = nc.dram_tensor("out", [NPC, H], F32, kind="ExternalOutput").ap()

    # internal DRAM
    hnbg = nc.dram_tensor("hnbg", [NPC, H], sdt).ap()  # AllGather input
    hnf = nc.dram_tensor("hnf", [NP, H], sdt, addr_space="Shared").ap()

    with tile.TileContext(nc) as tc:
        with tc.tile_pool(name="const", bufs=1) as cpool, \
             tc.tile_pool(name="colv", bufs=1) as colp:
            # constants
            ident = cpool.tile([128, 128], F32)
            make_identity(nc, ident[:])
            iota = cpool.tile([128, CE], sdt)
            nc.gpsimd.iota(iota[:], pattern=[[0, NCHUNK], [1, 128]], base=0,
                           channel_multiplier=0,
                           allow_small_or_imprecise_dtypes=True)
            w1_st = cpool.tile([H, H2 + 2], F32, tag="w1_st")
            nc.gpsimd.memset(w1_st[:, H2:H2 + 2], 0.0)
            nc.sync.dma_start(w1_st[:, 0:H2], W1[:])
            nc.vector.tensor_reduce(w1_st[:, H2:H2 + 1], w1_st[:, 0:H2],
                                    mybir.AxisListType.X, OP.add)
            w1_sb = cpool.tile([H, H2 + 2], F32R, tag="w1_sb")
            nc.scalar.copy(w1_sb[:], w1_st[:])
            w2_st = cpool.tile([H2 // 2, 2 * H], F32, tag="w2_st")
            nc.sync.dma_start(w2_st[:, 0:H], W2[0:H, :])
            nc.sync.dma_start(w2_st[:, H:2 * H], W2[H:H2, :])
            w2_sb = cpool.tile([H2 // 2, 2 * H], F32R, tag="w2_sb")
            nc.scalar.copy(w2_sb[:], w2_st[:])
            w2a_sb = w2_sb[:, 0:H]
            w2b_sb = w2_sb[:, H:2 * H]

            # scalar temperature columns
            t_c1 = colp.tile([1, 1], F32)
            nc.sync.dma_start(t_c1[:], t_sc[:])
            t_c = colp.tile([128, 1], F32)
            nc.gpsimd.partition_broadcast(t_c[:], t_c1[:])
            teps_c = colp.tile([128, 1], F32)
            nc.vector.tensor_scalar_mul(teps_c[:], t_c[:], float(EPS))
            etc_c = colp.tile([128, 1], F32)  # exp(t*eps)
            nc.scalar.activation(etc_c[:], teps_c[:], AF.Exp)
            bneps_r = colp.tile([1, 1], F32)
            nc.gpsimd.memset(bneps_r[:], float(BN_EPS))
            lneps_c = colp.tile([128, 1], F32)
            nc.gpsimd.memset(lneps_c[:], float(LN_EPS))
            zeros_h2 = cpool.tile([128, H2], F32, tag="zeros_h2")
            nc.gpsimd.memset(zeros_h2[:], 0.0)
            ones_c = colp.tile([1, 128], F32)
            nc.gpsimd.memset(ones_c[:], 1.0)
            # R16[k, m] = (m % 16 == k): 16->128 partition replication matrix
            if not cfg.fat_gidx:
                m16 = colp.tile([16, 128], F32, tag="m16")
                nc.gpsimd.iota(m16[:], pattern=[[0, 8], [1, 16]], base=0,
                               channel_multiplier=0,
                               allow_small_or_imprecise_dtypes=True)
                k16 = colp.tile([16, 1], F32, tag="k16")
                nc.gpsimd.iota(k16[:], pattern=[[0, 1]], base=0,
                               channel_multiplier=1,
                               allow_small_or_imprecise_dtypes=True)
                r16f = colp.tile([16, 128], F32, tag="r16f")
                nc.vector.tensor_scalar(r16f[:], m16[:], k16[:], None,
                                        OP.is_equal)
                r16b = colp.tile([16, 128], BF16, tag="r16b")
                nc.vector.tensor_copy(r16b[:], r16f[:])

            # bn affine rows: a = bnw / sqrt(bnv + eps); c = bnb - bnm * a
            bnw_r = colp.tile([1, H], F32, tag="bnw_r")
            nc.sync.dma_start(bnw_r[:], bnw[:])
            bnb_r = colp.tile([1, H], F32, tag="bnb_r")
            nc.sync.dma_start(bnb_r[:], bnb[:])
            bnm_r = colp.tile([1, H], F32, tag="bnm_r")
            nc.sync.dma_start(bnm_r[:], bnm[:])
            bnv_r = colp.tile([1, H], F32, tag="bnv_r")
            nc.sync.dma_start(bnv_r[:], bnv[:])
            lv_r = colp.tile([1, H], F32, tag="lv_r")
            nc.scalar.activation(lv_r[:], bnv_r[:], AF.Ln, bias=bneps_r[:])
            rs_r = colp.tile([1, H], F32, tag="rs_r")
            nc.scalar.activation(rs_r[:], lv_r[:], AF.Exp, scale=-0.5)
            a_r = colp.tile([1, H], F32, tag="a_r")
            nc.vector.tensor_mul(a_r[:], bnw_r[:], rs_r[:])
            ma_r = colp.tile([1, H], F32, tag="ma_r")
            nc.vector.tensor_mul(ma_r[:], bnm_r[:], a_r[:])
            c_r = colp.tile([1, H], F32, tag="c_r")
            nc.vector.tensor_sub(c_r[:], bnb_r[:], ma_r[:])

            # broadcast [1, W] rows to [128, W] via ones-matmul
            with tc.tile_pool(name="bc_ps", bufs=2, space="PSUM") as bcps:
                def bcast(row_ap, width, nm):
                    ps = bcps.tile([128, width], F32, tag=f"bc_{nm}")
                    sb = cpool.tile([128, width], F32, tag=f"bcsb_{nm}")
                    nc.tensor.matmul(ps[:], lhsT=ones_c[:], rhs=row_ap,
                                     start=True, stop=True)
                    nc.scalar.copy(sb[:], ps[:])
                    return sb

                a_b = bcast(a_r[:], H, "a")
                c_b = bcast(c_r[:], H, "c")

                def bcast_dram(dram_row, width, nm):
                    row = colp.tile([1, width], F32, tag=f"bcrow_{nm}")
                    nc.sync.dma_start(row[:], dram_row)
                    return bcast(row[:], width, nm)

                b1_b = bcast_dram(b1[:], H2, "b1") if cfg.apply_b1 else None
                b2_b = bcast_dram(b2[:], H, "b2") if cfg.apply_b2 else None
                lnw_b = (bcast_dram(lnw[:], H2, "lnw")
                         if cfg.apply_ln_affine else None)
                lnb_b = (bcast_dram(lnb[:], H2, "lnb")
                         if cfg.apply_ln_affine else None)

            # persistent per-core node data (rows layout, on partitions)
            xall = cpool.tile([128, NPC], F32, tag="xall")
            hnall = cpool.tile([128, NPC], F32, tag="hnall")

            # ---- phase A: hn = relu(a*h + c) per node tile, rows layout ----
            with tc.tile_pool(name="pa", bufs=4) as pa:
                for t in range(NT):
                    sl = slice(t * 128, (t + 1) * 128)
                    nc.sync.dma_start(xall[:, sl], h_rows[sl, :])
                    u = pa.tile([128, 128], F32, tag="u")
                    nc.vector.tensor_mul(u[:], xall[:, sl], a_b[:])
                    v = pa.tile([128, 128], F32, tag="v")
                    nc.vector.tensor_add(v[:], u[:], c_b[:])
                    nc.scalar.activation(hnall[:, sl], v[:], AF.Relu)
                    hng = pa.tile([128, 128], sdt, tag="hng")
                    nc.vector.tensor_copy(hng[:], hnall[:, sl])
                    nc.scalar.dma_start(hnbg[sl, :], hng[:])

            # ---- AllGather hn ----
            if cfg.skip_collective:
                nc.sync.dma_start(hnf[0:NPC, :], hnbg[:])
            else:
                nc.gpsimd.collective_compute(
                    "AllGather",
                    OP.bypass,
                    ins=[hnbg[:]],
                    outs=[hnf[:]],
                    replica_groups=[list(range(cfg.n_cores))],
                )

            # ---- phase B: per node tile ----
            with tc.tile_pool(name="stream", bufs=cfg.stream_bufs) as sp, \
                 tc.tile_pool(name="small", bufs=cfg.small_bufs) as smp, \
                 tc.tile_pool(name="ps_nd", bufs=2, space="PSUM") as ps_nd, \
                 tc.tile_pool(name="ps_tr", bufs=2, space="PSUM") as ps_tr, \
                 tc.tile_pool(name="ps_y", bufs=1, space="PSUM") as ps_y:
              qn = [0]
              for rep in range(cfg.hw_repeat):
                for t in range(NT):
                    sl = slice(t * 128, (t + 1) * 128)
                    # inputs for this tile
                    dsl = smp.tile([128, NCHUNK], F32, tag="dsl")
                    nc.scalar.dma_start(dsl[:], dst_sl[t])
                    ixt = smp.tile([128, CE // 16], I16, tag="ixt")
                    if cfg.fat_gidx:
                        nc.scalar.dma_start(ixt[:], gidx[t])
                    else:
                        # replicate the 16-partition idx wrap to 128 parts
                        # (byte-split so the bf16 matmul stays exact)
                        X16 = CE // 16
                        ix16 = smp.tile([16, 2 * X16], I16, tag="ix16")
                        nc.scalar.dma_start(ix16[:], gidx[t])
                        ixbf = smp.tile([16, 2 * X16], BF16, tag="ixbf")
                        nc.vector.tensor_copy(ixbf[:], ix16[:])
                        ixp = ps_y.tile([128, 2 * X16], F32, tag="ixp")
                        nc.tensor.matmul(ixp[:], lhsT=r16b[:], rhs=ixbf[:],
                                         start=True, stop=True)
                        hi_f = smp.tile([128, X16], F32, tag="hi_f")
                        nc.vector.tensor_scalar_mul(hi_f[:], ixp[:, X16:],
                                                    256.0)
                        nc.vector.tensor_add(ixt[:], hi_f[:], ixp[:, 0:X16])
                    ea = sp.tile([128, CE], sdt, tag="ea")
                    nc.sync.dma_start(ea[:], eattr[t])
                    hs = sp.tile([128, CE], sdt, tag="hs")
                    hs3 = hs[:].rearrange("p (j c) -> p j c", c=128)
                    MAXC = 8  # <=1024 idxs per call (SWDGE ring limit)

                    def gather_calls(c0, c1, tab):
                        for a in range(c0, c1, MAXC):
                            b = min(a + MAXC, c1)
                            nc.gpsimd.dma_gather(
                                out_ap=hs3[:, a:b, :],
                                in_ap=tab,
                                idxs_ap=ixt[:, (a * 128) // 16:(b * 128) // 16],
                                num_idxs=(b - a) * 128,
                                num_idxs_reg=(b - a) * 128,
                                elem_size=H,
                                queue_num=qn[0] % cfg.n_queues,
                            )
                            qn[0] += 1
                    if "gather" not in cfg.ablate:
                        gather_calls(0, cfg.C_LO, hnf[:])
                        if cfg.C_HI:
                            gather_calls(cfg.C_LO, NCHUNK, hnf[cfg.SPLIT:NP, :])
                    else:
                        nc.sync.dma_start(hs[:], eattr[t])
                    # s = hs + ea
                    s = sp.tile([128, CE], sdt, tag="s")
                    gp = cfg.add_gp_chunks * 128
                    if gp:
                        nc.gpsimd.tensor_add(s[:, CE - gp:], hs[:, CE - gp:],
                                             ea[:, CE - gp:])
                    if gp < CE:
                        nc.vector.tensor_add(s[:, :CE - gp], hs[:, :CE - gp],
                                             ea[:, :CE - gp])
                    s3 = s[:].rearrange("p (j c) -> p j c", c=128)
                    # exmex: [ex_j | mex_j] interleaved per chunk
                    # ex = max(exp(t*s + t*eps), exp(t*eps)) = exp(t*(relu(s)+eps))
                    # mex = relu(s) * ex   (num = mex_sum + eps*den, folded later)
                    exmex = sp.tile([128, 2 * CE], sdt, tag="exmex")
                    em3 = exmex[:].rearrange("p (j c) -> p j c", c=256)
                    ex_v = em3[:, :, 0:128]
                    mex_v = em3[:, :, 128:256]
                    nc.scalar.activation(ex_v, s3, AF.Exp,
                                         scale=t_c[:], bias=teps_c[:])
                    nc.vector.tensor_scalar(ex_v, ex_v, etc_c[:], None, OP.max)
                    mgp = min(cfg.mex_gp_chunks, NCHUNK)
                    if mgp:
                        nc.gpsimd.scalar_tensor_tensor(
                            mex_v[:, NCHUNK - mgp:, :],
                            s3[:, NCHUNK - mgp:, :], 0.0,
                            ex_v[:, NCHUNK - mgp:, :], OP.max, OP.mult)
                    if mgp < NCHUNK:
                        nc.vector.scalar_tensor_tensor(
                            mex_v[:, :NCHUNK - mgp, :], s3[:, :NCHUNK - mgp, :],
                            0.0, ex_v[:, :NCHUNK - mgp, :], OP.max, OP.mult)
                    # indicator A[e, n] = (dst_slot[e] == iota_n)
                    A = sp.tile([128, CE], sdt, tag="A")
                    A3 = A[:].rearrange("p (j c) -> p j c", c=128)
                    i3 = iota[:].rearrange("p (j c) -> p j c", c=128)
                    kg = min(cfg.iseq_gp_chunks, NCHUNK)
                    if cfg.iseq_dve_ts:
                        for j in range(NCHUNK - kg):
                            nc.vector.tensor_scalar(
                                A3[:, j, :], i3[:, j, :],
                                dsl[:, j:j + 1], None, OP.is_equal)
                    else:
                        kd = NCHUNK - kg
                        if kd:
                            d_b = dsl[:, 0:kd].unsqueeze(2).to_broadcast(
                                [128, kd, 128])
                            nc.vector.tensor_tensor(A3[:, 0:kd, :],
                                                    i3[:, 0:kd, :],
                                                    d_b, OP.is_equal)
                    for j in range(NCHUNK - kg, NCHUNK):
                        nc.gpsimd.tensor_scalar(A3[:, j, :], i3[:, j, :],
                                                dsl[:, j:j + 1], None,
                                                OP.is_equal)
                    # accumulate [den | num']
                    nd = ps_nd.tile([128, 256], F32, tag="nd")
                    for j in range(NCHUNK):
                        nc.tensor.matmul(
                            nd[:],
                            lhsT=A[:, j * 128:(j + 1) * 128],
                            rhs=exmex[:, j * 256:(j + 1) * 256],
                            start=(j == 0), stop=(j == NCHUNK - 1),
                        )
                    # agg = (num' + eps*den)/(den + 1e-16); x_r = agg + hn_row
                    den = nd[:, 0:128]
                    d1 = smp.tile([128, 128], F32, tag="d1")
                    nc.vector.tensor_scalar_add(d1[:], den, float(DEN_EPS))
                    num = smp.tile([128, 128], F32, tag="num")
                    nc.vector.scalar_tensor_tensor(num[:], d1[:], float(EPS),
                                                   nd[:, 128:256],
                                                   OP.mult, OP.add)
                    rden = smp.tile([128, 128], F32, tag="rden")
                    nc.vector.reciprocal_approx_fast(rden[:], d1[:])
                    agg = smp.tile([128, 128], F32, tag="agg")
                    nc.vector.tensor_mul(agg[:], num[:], rden[:])
                    aggx = smp.tile([128, 128], F32, tag="aggx")
                    nc.vector.tensor_add(aggx[:], agg[:], hnall[:, sl])
                    # MLP
                    tps = ps_tr.tile([128, 128], F32, tag="tps")
                    nc.tensor.transpose(tps[:], aggx[:], ident[:])
                    aggxT = smp.tile([128, 128], F32R, tag="aggxT")
                    nc.scalar.copy(aggxT[:], tps[:])
                    y1 = ps_y.tile([128, H2 + 2], F32, tag="y1")
                    nc.tensor.matmul(y1[:], lhsT=aggxT[:], rhs=w1_sb[:],
                                     start=True, stop=True)
                    if cfg.apply_b1:
                        y1s = smp.tile([128, H2], F32, tag="y1s")
                        nc.vector.tensor_add(y1s[:], y1[:, 0:H2], b1_b[:])
                        sums = smp.tile([128, 1], F32, tag="sums")
                        nc.vector.tensor_reduce(sums[:], y1s[:],
                                                mybir.AxisListType.X, OP.add)
                        y1v = y1s[:]
                    else:
                        sums = y1[:, H2:H2 + 1]
                        y1v = y1[:, 0:H2]
                    sq = smp.tile([128, H2], F32, tag="sq")
                    sumsq = smp.tile([128, 1], F32, tag="sumsq")
                    nc.scalar.activation(sq[:], y1v, AF.Square,
                                         accum_out=sumsq[:])
                    mu = smp.tile([128, 1], F32, tag="mu")
                    nc.vector.tensor_scalar_mul(mu[:], sums, 1.0 / H2)
                    msq = smp.tile([128, 1], F32, tag="msq")
                    nc.vector.tensor_mul(msq[:], mu[:], mu[:])
                    var = smp.tile([128, 1], F32, tag="var")
                    nc.vector.scalar_tensor_tensor(var[:], sumsq[:], 1.0 / H2,
                                                   msq[:], OP.mult, OP.subtract)
                    # rstd = (var+eps)^-0.5 = exp(-0.5*ln(var+eps)) -- stays in
                    # the natural_log_exp ACT table set (no per-tile reload)
                    lv = smp.tile([128, 1], F32, tag="lv")
                    nc.scalar.activation(lv[:], var[:], AF.Ln,
                                         bias=lneps_c[:])
                    rstd = smp.tile([128, 1], F32, tag="rstd")
                    nc.scalar.activation(rstd[:], lv[:], AF.Exp, scale=-0.5)
                    z = smp.tile([128, H2], F32, tag="z")
                    nc.vector.tensor_scalar(z[:], y1v, mu[:], rstd[:],
                                            OP.subtract, OP.mult)
                    if cfg.apply_ln_affine:
                        nc.vector.tensor_mul(z[:], z[:], lnw_b[:])
                        nc.vector.tensor_add(z[:], z[:], lnb_b[:])
                    yr = smp.tile([128, H2], F32, tag="yr")
                    nc.scalar.activation(yr[:], z[:], AF.Relu)
                    # transpose both halves for GEMM2
                    o_ps = ps_tr.tile([128, H], F32, tag="o_ps")
                    for half in range(2):
                        tph = ps_tr.tile([128, 128], F32, tag="tps")
                        nc.tensor.transpose(
                            tph[:], yr[:, half * 128:(half + 1) * 128],
                            ident[:])
                        yT = smp.tile([128, 128], F32R, tag="yT")
                        nc.scalar.copy(yT[:], tph[:])
                        nc.tensor.matmul(
                            o_ps[:], lhsT=yT[:],
                            rhs=(w2a_sb if half == 0 else w2b_sb),
                            start=(half == 0), stop=(half == 1))
                    osb = smp.tile([128, 128], F32, tag="osb")
                    nc.vector.tensor_add(osb[:], o_ps[:], xall[:, sl])
                    if cfg.apply_b2:
                        nc.vector.tensor_add(osb[:], osb[:], b2_b[:])
                    nc.scalar.dma_start(out[sl, :], osb[:])

    nc.compile()
    return nc


# ---------------- host-side prep ----------------

try:
    import ml_dtypes
    ml_bf16 = ml_dtypes.bfloat16
except ImportError:
    ml_bf16 = np.float32


def host_prep(h, edge_index, edge_attr, bn_weight, bn_bias, bn_mean, bn_var,
              t, W1, b1, ln_weight, ln_bias, W2, b2, n_cores=8, split=32768):
    """Returns (cfg, in_maps, meta). Pure data movement + layout."""
    h = np.asarray(h, np.float32)
    edge_index = np.asarray(edge_index).astype(np.int64)
    edge_attr = np.asarray(edge_attr, np.float32)
    N, H = h.shape
    E = edge_index.shape[1]

    NT = int(np.ceil(N / (n_cores * 128)))
    NPC = NT * 128
    NP = NPC * n_cores
    n_tiles_all = n_cores * NT
    SPLIT = split

    src = edge_index[0]
    dst = edge_index[1]

    # degree-balanced node -> (tile, slot) assignment: sort nodes by in-degree
    # descending, deal round-robin across all tiles. perm_pos[n] = global slot.
    deg = np.bincount(dst, minlength=N).astype(np.int64)
    order_nodes = np.argsort(-deg, kind="stable")
    perm_pos = np.empty(N, np.int64)
    ranks = np.arange(N, dtype=np.int64)
    perm_pos[order_nodes] = (ranks % n_tiles_all) * 128 + ranks // n_tiles_all
    # node_of_slot: inverse map (only for slots holding a real node)
    node_of_slot = np.full(NP, -1, np.int64)
    node_of_slot[perm_pos] = np.arange(N, dtype=np.int64)

    p_src = perm_pos[src]
    p_dst = perm_pos[dst]
    gtile = p_dst // 128          # global tile id in [0, n_tiles_all)
    slot = p_dst % 128

    # pick the lo/hi table split (int16 gather idx limit) minimizing padding
    best = None
    s_min = max(1, NP - 32767)
    for S in range(s_min, 32769, 512):
        lo = np.bincount(gtile[p_src < S], minlength=n_tiles_all)
        hi = np.bincount(gtile[p_src >= S], minlength=n_tiles_all)
        cl = max(1, int(np.ceil(lo.max() / 128)))
        ch = int(np.ceil(hi.max() / 128))
        if best is None or cl + ch < best[0]:
            best = (cl + ch, S, cl, ch)
    _, SPLIT, C_LO, C_HI = best
    hi_flag = (p_src >= SPLIT).astype(np.int64)

    order = np.lexsort((p_src, hi_flag, gtile))
    src_s = p_src[order]
    gt_s = gtile[order]
    slot_s = slot[order]
    hi_s = hi_flag[order]

    NCHUNK = C_LO + C_HI
    CAP = NCHUNK * 128

    # logical position of each edge within its tile: lo edges from 0,
    # hi edges from C_LO*128
    grp = gt_s * 2 + hi_s
    grp_starts = np.zeros(2 * n_tiles_all, np.int64)
    cnt2 = np.bincount(grp, minlength=2 * n_tiles_all)
    np.cumsum(cnt2[:-1], out=grp_starts[1:])
    rank = np.arange(E, dtype=np.int64) - grp_starts[grp]
    logical = np.where(hi_s == 0, rank, C_LO * 128 + rank)

    p_idx = logical % 128
    j_idx = logical // 128
    apply_b1 = not np.allclose(np.asarray(b1), 0.0)
    apply_b2 = not np.allclose(np.asarray(b2), 0.0)
    apply_ln = not (np.allclose(np.asarray(ln_weight), 1.0)
                    and np.allclose(np.asarray(ln_bias), 0.0))

    cfg = Cfg(n_cores=n_cores, H=H, NT=NT, C_LO=C_LO, C_HI=C_HI, SPLIT=SPLIT,
              apply_b1=apply_b1, apply_b2=apply_b2, apply_ln_affine=apply_ln)

    dst_pad = np.full((n_tiles_all, 128, NCHUNK), -1.0, np.float32)
    dst_pad[gt_s, p_idx, j_idx] = slot_s.astype(np.float32)
    ea_pad = np.zeros((n_tiles_all, 128, NCHUNK, H), ml_bf16)
    ea_pad[gt_s, p_idx, j_idx, :] = edge_attr[order].astype(ml_bf16)
    ea_pad = ea_pad.reshape(n_tiles_all, 128, NCHUNK * H)

    # int16 gather indices, wrapped in 16 partitions
    gidx16 = np.zeros((n_tiles_all, 16, CAP // 16), np.int16)
    idx_val = np.where(hi_s == 0, src_s, src_s - SPLIT).astype(np.int16)
    gidx16[gt_s, logical % 16, logical // 16] = idx_val
    gidx2 = np.concatenate([gidx16 & 255, gidx16 >> 8], axis=2)

    h_pad = np.zeros((NP, H), np.float32)
    h_pad[perm_pos] = h

    com = dict(
        bnw=np.asarray(bn_weight, np.float32).reshape(1, H),
        bnb=np.asarray(bn_bias, np.float32).reshape(1, H),
        bnm=np.asarray(bn_mean, np.float32).reshape(1, H),
        bnv=np.asarray(bn_var, np.float32).reshape(1, H),
        t_sc=np.asarray(t, np.float32).reshape(1, 1),
        W1=np.asarray(W1, np.float32),
        W2=np.asarray(W2, np.float32),
    )
    if apply_b1:
        com["b1"] = np.asarray(b1, np.float32).reshape(1, 2 * H)
    if apply_b2:
        com["b2"] = np.asarray(b2, np.float32).reshape(1, H)
    if apply_ln:
        com["lnw"] = np.asarray(ln_weight, np.float32).reshape(1, 2 * H)
        com["lnb"] = np.asarray(ln_bias, np.float32).reshape(1, 2 * H)

    in_maps = []
    for c in range(n_cores):
        sl = slice(c * NPC, (c + 1) * NPC)
        m = dict(com)
        m["h_rows"] = np.ascontiguousarray(h_pad[sl])
        m["gidx"] = np.ascontiguousarray(gidx2[c * NT:(c + 1) * NT])
        m["dst_sl"] = np.ascontiguousarray(dst_pad[c * NT:(c + 1) * NT])
        m["eattr"] = np.ascontiguousarray(ea_pad[c * NT:(c + 1) * NT])
        in_maps.append(m)

    meta = dict(N=N, NPC=NPC, perm_pos=perm_pos)
    return cfg, in_maps, meta


def assemble_output(results, meta):
    N, NPC = meta["N"], meta["NPC"]
    full = np.concatenate([r["out"] for r in results], axis=0)
    return full[meta["perm_pos"]].astype(np.float32)


# ---------------- harness entrypoint ----------------

_cache = {}


def kernel(**inputs):
    """Full-input DeeperGCN layer on 8 NeuronCores; returns [N, H] float32."""
    cfg, in_maps, meta = host_prep(**{k: np.asarray(v)
                                      for k, v in inputs.items()}, n_cores=8)
    key = (cfg.NT, cfg.C_LO, cfg.C_HI, cfg.apply_b1, cfg.apply_b2,
           cfg.apply_ln_affine)
    nc = _cache.get(key)
    if nc is None:
        nc = build_gcn(cfg)
        _cache[key] = nc

    from concourse.bass_utils import run_bass_kernel_spmd
    res = run_bass_kernel_spmd(nc, in_maps, core_ids=list(range(cfg.n_cores)))
    return assemble_output(res.results, meta)

